# revision 10
# baseline (speedup 1.0000x reference)
"""Multi-head attention (b=2, n=2048, dim=1024, h=16, fp32) on 8 TRN2 NeuronCores.

Sharding: 2 batches x 4 head-groups (4 heads each). Each core computes, for its
batch element and 4 heads: QKV projection, softmax attention, and a partial
output projection (W_out rows of its heads). Host sums the 4 partials per batch
and adds the bias.

v2: fp8(e4m3) DoubleRow matmuls for the QKV projection and PV product
(contraction 256/instr at 0.5 cyc/col). The S=QK^T matmul stays fp16
(contraction 64 is PE-output-rate bound; dtype doesn't help), and the output
projection stays fp16 (DR would force 64-partition psums there, doubling the
evacuation cost on DVE for only ~7us of PE win).

Hardware constraint: DoubleRow matmuls must write psum at partition base 0, so
all DR outputs use [64, ...] psum tiles and the Q^T/K^T tiles are stored
[64, 2(head-in-pair), 512] instead of [128, 512].

Scale management (fp8 needs O(1) values everywhere):
  host:   Wqkv' = 32*Wqkv (fp8), x as-is (fp8), Wout as-is (fp16)
  device: qt = kt = (32Q)*2^-5.5 (fp16)  =>  S_psum = 16*x where x = S/sqrt(dim)
          is the softmax argument (|x| <~ 0.5, no max-subtraction needed)
          e = exp(S_psum/16) in fp8
          vt = (32V)*0.5 = 16V in fp8, trailing ones column per head so PV row
          DH accumulates the softmax denominator D
          on = (16*O_num)/D = 16*O in fp16
          out_psum = on @ Wout = 16*out; evac scales by 1/16 -> fp16
"""

import os
import numpy as np
import ml_dtypes
from contextlib import ExitStack

import concourse.bass as bass
import concourse.mybir as mybir
import concourse.tile as tile
from concourse import bacc
from concourse.bass import ts
from concourse.bass_utils import run_bass_kernel_spmd

F32 = mybir.dt.float32
F16 = mybir.dt.float16
F8 = mybir.dt.float8e4
NP_F8 = ml_dtypes.float8_e4m3
DR = mybir.MatmulPerfMode.DoubleRow

N_CORES = 8
HEADS = 16
DH = 64  # head dim

QK_SCALE = 2.0 ** -5.5  # qt = kt = 32Q * 2^-5.5 => S_psum = 16 * softmax-arg
EXP_SCALE = 1.0 / 16.0
V_SCALE = 0.5           # vt = 32V * 0.5 = 16V
O_SCALE = 1.0 / 16.0    # out_psum = 16*O @ Wout = 16*out


class Cfg:
    def __init__(self, n, dim, hg):
        self.n = n                    # sequence length
        self.dim = dim                # model dim
        self.hg = hg                  # heads per core
        self.kc = dim // 128          # dim chunks of 128
        self.kc2 = dim // 256         # dim chunks of 256 (DoubleRow)
        self.nqb = n // 512           # query blocks of 512
        self.qb = 512
        self.nkc = n // 128           # key chunks of 128
        self.nc2 = n // 256           # key chunk-pairs of 256
        self.pairs = hg // 2
        self.shard = hg * DH          # qkv shard columns per section (256)
        self.np_dt = NP_F8


FULL = Cfg(2048, 1024, 4)


def build_kernel(tc, ctx, cfg, x8, wq, wk, wv, wo, out):
    nc = tc.nc
    P = 128
    KC2, NQB, QB, NC2, PAIRS = cfg.kc2, cfg.nqb, cfg.qb, cfg.nc2, cfg.pairs
    SH = cfg.shard

    wpool = ctx.enter_context(tc.tile_pool(name="w", bufs=1))
    wq_sb = wpool.tile([P, KC2, 2, SH], F8, tag="wq", name="wq_sb")
    wk_sb = wpool.tile([P, KC2, 2, SH], F8, tag="wk", name="wk_sb")
    wv_sb = wpool.tile([P, KC2, 2, SH], F8, tag="wv", name="wv_sb")
    wo_sb = wpool.tile([P, 2, cfg.dim], F16, tag="wo", name="wo_sb")

    xpool = ctx.enter_context(tc.tile_pool(name="x", bufs=1))
    x_sb = xpool.tile([P, KC2, 2, cfg.n], F8, tag="x", name="x_sb")

    per = ctx.enter_context(tc.tile_pool(name="per", bufs=1))
    qt = {}  # (pair, qb) -> [64, 2, 512] fp16: [dim-in-head, head-in-pair, query]
    kt = {}
    vt = {}  # c2 -> [128, 2, 4*64+1] fp8: [key, chunk-in-pair, head-dims | ones]
    on = {}  # qb -> [128, 2, 512] fp16: [pair-dims, pair, queries] = 16*O
    VW = 320  # ones col at 256; dual-fp8 ldweights needs sub-tile stride % 64 == 0
    for g in range(PAIRS):
        for b in range(NQB):
            qt[g, b] = per.tile([DH, 2, QB], F16, tag=f"qt{g}_{b}", name=f"qt{g}_{b}")
            kt[g, b] = per.tile([DH, 2, QB], F16, tag=f"kt{g}_{b}", name=f"kt{g}_{b}")
    for c2 in range(NC2):
        vt[c2] = per.tile([P, 2, VW], F8, tag=f"v{c2}", name=f"v{c2}")
    for b in range(NQB):
        on[b] = per.tile([P, 2, QB], F16, tag=f"on{b}", name=f"on{b}")

    paQ = ctx.enter_context(tc.tile_pool(name="paQ", bufs=2, space="PSUM"))
    psS = ctx.enter_context(tc.tile_pool(name="psS", bufs=2, space="PSUM"))
    psO = ctx.enter_context(tc.tile_pool(name="psO", bufs=1, space="PSUM"))
    epool = ctx.enter_context(tc.tile_pool(name="e", bufs=6))
    npool = ctx.enter_context(tc.tile_pool(name="nrm", bufs=2))
    copool = ctx.enter_context(tc.tile_pool(name="co", bufs=6))

    # ---- DMA staging: K path first (kt tiles are emitted first)
    nc.sync.dma_start(wk_sb[:], wk[:])
    nc.sync.dma_start(x_sb[:, :, :, ts(0, QB)], x8[:, :, :, ts(0, QB)])
    nc.sync.dma_start(wq_sb[:], wq[:])
    for b in range(1, NQB):
        nc.sync.dma_start(x_sb[:, :, :, ts(b, QB)], x8[:, :, :, ts(b, QB)])
    nc.sync.dma_start(wv_sb[:], wv[:])

    def emit_qk(w_sb, dst, g, b):
        """Q^T/K^T tile [64, 2, 512] for pair g, block b via fp8 DoubleRow.
        mb = head-in-pair (64 W-columns each); DR dst must be partition 0."""
        for mb in range(2):
            ps = paQ.tile([DH, 512], F32, tag="pa", name="pa")
            for nb in range(2):
                for c2 in range(KC2):
                    nc.tensor.matmul(
                        ps[:, ts(nb, 256)],
                        lhsT=w_sb[:, c2, :, g * 128 + 64 * mb : g * 128 + 64 * mb + 64],
                        rhs=x_sb[:, c2, :, b * QB + 256 * nb : b * QB + 256 * nb + 256],
                        start=(c2 == 0),
                        stop=(c2 == KC2 - 1),
                        perf_mode=DR,
                    )
            nc.vector.tensor_scalar_mul(dst[g, b][:, mb, :], ps[:], QK_SCALE)

    def emit_v(c2):
        """V tile [128 keys, 2, 4*64+1] fp8 = 16*V, trailing shared ones col."""
        for i in range(2):
            c = 2 * c2 + i
            for mb in range(2):
                ps = paQ.tile([DH, 512], F32, tag="pa", name="pa")
                for c2w in range(KC2):
                    nc.tensor.matmul(
                        ps[:, 0:SH],
                        lhsT=x_sb[:, c2w, :, c * 128 + 64 * mb : c * 128 + 64 * mb + 64],
                        rhs=wv_sb[:, c2w, :, :],
                        start=(c2w == 0),
                        stop=(c2w == KC2 - 1),
                        perf_mode=DR,
                    )
                nc.vector.tensor_scalar_mul(
                    vt[c2][64 * mb : 64 * mb + 64, i, 0:SH],
                    ps[:, 0:SH],
                    V_SCALE,
                )
        nc.vector.memset(vt[c2][:, :, SH : SH + 1], 1.0)

    emitted = set()

    def attention(b, g, with_v=False, fillers=None, nfill=0):
        """One (query-block, pair) sweep; the two heads run sequentially so the
        denominator fits its own [1, 512] psum bank (DR dst must be base 0)."""
        done = 0
        for a in range(2):
            hl = 2 * g + a
            o_ps = psO.tile([DH, QB], F32, tag="o", name="o_ps")
            d_ps = psO.tile([1, QB], F32, tag="d", name="d_ps")
            for c2 in range(NC2):
                if with_v and a == 0:
                    emit_v(c2)
                if fillers and done < nfill and c2 in (2, 5):
                    key, fn = fillers.popleft()
                    fn()
                    emitted.add(key)
                    done += 1
                s_ps = psS.tile([P, 2, QB], F32, tag="s", name="s_ps")
                for i in range(2):
                    c = 2 * c2 + i
                    kb, pos = divmod(c * 128, QB)
                    nc.tensor.matmul(
                        s_ps[:, i, :],
                        lhsT=kt[g, kb][:, a, pos : pos + 128],
                        rhs=qt[g, b][:, a, :],
                        start=True,
                        stop=True,
                    )
                e2 = epool.tile([P, 2, QB], F8, tag="e", name="e2")
                nc.scalar.activation(
                    e2[:], s_ps[:], mybir.ActivationFunctionType.Exp, scale=EXP_SCALE
                )
                nc.tensor.matmul(
                    o_ps[:],
                    lhsT=vt[c2][:, :, hl * DH : (hl + 1) * DH],
                    rhs=e2[:],
                    start=(c2 == 0),
                    stop=(c2 == NC2 - 1),
                    perf_mode=DR,
                )
                nc.tensor.matmul(
                    d_ps[:],
                    lhsT=vt[c2][:, :, SH : SH + 1],
                    rhs=e2[:],
                    start=(c2 == 0),
                    stop=(c2 == NC2 - 1),
                    perf_mode=DR,
                )
            # normalize head a: D is already at psum partition 0
            oev = npool.tile([DH, QB], F32, tag="oev", name="oev")
            nc.vector.tensor_copy(oev[:], o_ps[:])
            drow = npool.tile([1, QB], F32, tag="drow", name="drow")
            nc.vector.tensor_copy(drow[:], d_ps[:])
            recip = npool.tile([1, QB], F32, tag="recip", name="recip")
            nc.vector.reciprocal_approx_fast(out=recip[:], in_=drow[:])
            bcast = npool.tile([DH, QB], F32, tag="bcast", name="bcast")
            nc.gpsimd.partition_broadcast(bcast[:], recip[:])
            nc.vector.tensor_tensor(
                on[b][64 * a : 64 * a + 64, g, :],
                oev[:],
                bcast[:],
                mybir.AluOpType.mult,
            )

    # ---- emission schedule (same shape as baseline): kt pair0 first, then
    # attention sweeps with projection fillers interleaved.
    from collections import deque
    from functools import partial

    for b in range(NQB):
        emit_qk(wk_sb, kt, 0, b)
    emit_qk(wq_sb, qt, 0, 0)
    nc.sync.dma_start(wo_sb[:], wo[:])

    pend = deque()
    for b in range(1, NQB):
        pend.append((("q", 0, b), partial(emit_qk, wq_sb, qt, 0, b)))
    if PAIRS > 1:
        for b in range(NQB):
            pend.append((("k", 1, b), partial(emit_qk, wk_sb, kt, 1, b)))
        for b in range(NQB):
            pend.append((("q", 1, b), partial(emit_qk, wq_sb, qt, 1, b)))

    def fill_one():
        key, fn = pend.popleft()
        fn()
        emitted.add(key)

    def require(keys):
        while pend and any(k not in emitted for k in keys):
            fill_one()

    def sweep(b, g, **kw):
        keys = [("q", g, b)] if (g, b) != (0, 0) else []
        keys += [("k", g, bb) for bb in range(NQB)] if g > 0 else []
        require(keys)
        attention(b, g, fillers=pend, nfill=kw.pop("nfill", 0), **kw)

    attention(0, 0, with_v=True, fillers=pend, nfill=3)
    for b in range(1, NQB):
        sweep(b, 0, nfill=2)

    def out_proj(bb, late):
        """fp16 out-projection: psum [128 queries, 512 cols], contraction over
        the group's 256 O-dims (2 slabs of 128)."""
        for tp in range(QB // 128):
            t0 = tp * 128
            for ch in range(cfg.dim // 512):
                ps = paQ.tile([P, 512], F32, tag="pa", name="pc")
                for i in range(2):
                    nc.tensor.matmul(
                        ps[:],
                        lhsT=on[bb][:, i, t0 : t0 + 128],
                        rhs=wo_sb[:, i, ts(ch, 512)],
                        start=(i == 0),
                        stop=(i == 1),
                    )
                ot = copool.tile([P, 512], F16, tag="ot", name="ot")
                if late:
                    nc.scalar.mul(ot[:], ps[:], O_SCALE)
                else:
                    nc.vector.tensor_scalar_mul(ot[:], ps[:], O_SCALE)
                nc.sync.dma_start(
                    out[bb * QB + t0 : bb * QB + t0 + 128, ts(ch, 512)],
                    ot[:],
                )

    if PAIRS > 1:
        for b in range(NQB):
            sweep(b, 1, nfill=1)
    while pend:
        fill_one()
    for b in range(NQB):
        out_proj(b, late=(b >= NQB - 2))


def build_program(cfg, num_devices=N_CORES):
    nc = bacc.Bacc("TRN2", target_bir_lowering=False, debug=False, num_devices=num_devices)
    P = 128
    x8 = nc.dram_tensor("x8", [P, cfg.kc2, 2, cfg.n], F8, kind="ExternalInput").ap()
    wq = nc.dram_tensor("wq", [P, cfg.kc2, 2, cfg.shard], F8, kind="ExternalInput").ap()
    wk = nc.dram_tensor("wk", [P, cfg.kc2, 2, cfg.shard], F8, kind="ExternalInput").ap()
    wv = nc.dram_tensor("wv", [P, cfg.kc2, 2, cfg.shard], F8, kind="ExternalInput").ap()
    wo = nc.dram_tensor("wo", [P, 2, cfg.dim], F16, kind="ExternalInput").ap()
    out = nc.dram_tensor("out", [cfg.n, cfg.dim], F16, kind="ExternalOutput").ap()
    with tile.TileContext(nc) as tc, ExitStack() as ctx:
        build_kernel(tc, ctx, cfg, x8, wq, wk, wv, wo, out)
    nc.compile()
    return nc


def shard_inputs(cfg, x, W_qkv, W_out, n_groups):
    """Build per-core input maps. Core c = (batch b, head-group g): c = b*n_groups + g."""
    b_sz = x.shape[0]
    dim, sh = cfg.dim, cfg.shard
    x8s = []
    for b in range(b_sz):
        xt = np.ascontiguousarray(
            x[b].T.reshape(cfg.kc, 128, cfg.n).transpose(1, 0, 2)
        ).reshape(128, cfg.kc2, 2, cfg.n)
        x8s.append(xt.astype(NP_F8))

    def wcols(w):  # [dim, 256] -> [128, kc2, 2, 256] fp8 with 32x scale
        return np.ascontiguousarray(
            (32.0 * w).reshape(cfg.kc2, 2, 128, w.shape[1]).transpose(2, 0, 1, 3)
        ).astype(NP_F8)

    in_maps = []
    for b in range(b_sz):
        for g in range(n_groups):
            wq = wcols(W_qkv[:, sh * g : sh * (g + 1)])
            wk = wcols(W_qkv[:, dim + sh * g : dim + sh * (g + 1)])
            wv = wcols(W_qkv[:, 2 * dim + sh * g : 2 * dim + sh * (g + 1)])
            wo = np.ascontiguousarray(
                W_out[sh * g : sh * (g + 1), :].reshape(2, 128, dim).transpose(1, 0, 2)
            ).astype(np.float16)
            in_maps.append({"x8": x8s[b], "wq": wq, "wk": wk, "wv": wv, "wo": wo})
    return in_maps


_NC_CACHE = {}


def kernel(x, W_qkv, W_out, b_out):
    x = np.asarray(x, np.float32)
    W_qkv = np.asarray(W_qkv, np.float32)
    W_out = np.asarray(W_out, np.float32)
    b_out = np.asarray(b_out, np.float32)
    cfg = FULL
    bsz = x.shape[0]
    n_groups = N_CORES // bsz

    if "nc" not in _NC_CACHE:
        _NC_CACHE["nc"] = build_program(cfg)
    nc = _NC_CACHE["nc"]

    in_maps = shard_inputs(cfg, x, W_qkv, W_out, n_groups)
    res = run_bass_kernel_spmd(nc, in_maps, list(range(N_CORES)))

    out = np.zeros((bsz, cfg.n, cfg.dim), np.float32)
    for b in range(bsz):
        for g in range(n_groups):
            out[b] += res.results[b * n_groups + g]["out"].astype(np.float32)
        out[b] += b_out
    return out
